# revision 20
# baseline (speedup 1.0000x reference)
"""Trainium2 Bass kernel for CustomAttention (ViT-style windowed attention).

Math (per batch element):
  qkv = x @ qkv_w.T + qkv_b            -> q, k, v  [H=12 heads, D=64]
  s   = (q * D^-0.5) @ k.T             masked by a fixed 24x24-grid window
  attn = softmax(s)                    (CLS row/col always attended)
  out  = attn @ v                      -> concat heads -> @ proj_w.T + proj_b

Sharding: data-parallel over batch across 8 cores (4 images/core).

v2 design notes (vs the dense f32r baseline):
  - all matmul operands bf16 (rate-1 at any free-dim size, FWL weight loads,
    half the DMA/SBUF traffic); psum accumulation stays fp32.
  - BANDED sparse attention: the window mask is a +-25 band around the
    diagonal (plus dense CLS row/col).  Scores/exp/mask/attn@v for key tiles
    1..4 are computed only on the band query range plus a 1-wide CLS-query
    column; key tile 0 (contains the CLS key) stays dense.  This cuts the
    es area from 5*578 to ~1207 cols/head on every engine that touches it.
  - per head ONE packed psum score tile [128, 1294] (3 banks); exp + mask
    split at the dense/banded boundary so subtile deps let the next head's
    score matmuls reuse the psum tile earlier (the layout gap only ever
    holds values no matmul reads).
  - attn@v accumulates into PA [65,512] + PB(pair-shared) [65,130] psum with
    per-element has_written semantics (start=True only on the kt0 matmuls);
    v carries an interleaved ones column so row 64 is the softmax denominator.
  - head epilogue: one [65,577] bf16 copy pair to SBUF, then DMA row 0..63 to
    the channel-major oc tile (partition shift for odd heads) and row 64 to a
    DRAM denominator tile.  Denominators are batched per 6 heads as [64,60]
    full-lane reciprocals (the baseline's [6,578] reciprocal was 3.7us each).
  - qk bias+scale folded into the scalar-engine psum evacuation
    (Identity activation with per-partition bias).
  - psum pools are split per phase (scores / attn-out / qk+v / proj) so the
    next image's qk+v matmuls are not ring-serialized behind proj waiting on
    the denominator DMA round trip; oc/denominator DMAs issue from sync,
    reciprocal-broadcast DMAs from the gpsimd queue (measured fastest mix).
  - x(b+1) prefetches two pairs early and v tiles are double-buffered so the
    next image's v matmuls fill the image-boundary gap; each proj is emitted
    one image late so it back-fills attention-phase and boundary stalls.
"""

import ml_dtypes
import numpy as np

import concourse.bass as bass
import concourse.mybir as mybir
from concourse import bacc
from concourse.bass_utils import run_bass_kernel_spmd
from concourse.tile import TileContext

B, N, C = 32, 577, 768
H, D = 12, 64
NCORES = 8
BPC = B // NCORES            # images per core
NP = N + 1                   # padded token stride in xT
T = BPC * N
TP = BPC * NP
SCALE = float(D) ** -0.5
F32 = mybir.dt.float32
BF16 = mybir.dt.bfloat16
P = 128
CT = C // P                                              # 6 channel tiles
KT = [(0, 128), (128, 128), (256, 128), (384, 128), (512, 65)]
BAND = [(103, 178), (231, 178), (359, 178), (487, 90)]   # kt 1..4 (q0, width)

# packed es column layout shared by the psum score tile, the bf16 es tile
# and the mask tile.  Bank boundaries (512 f32 cols) respected per matmul.
ES0A, ES0B = 0, 512          # kt0 dense: queries [0,512) and [512,577)
Q0C = [None, 577, 756, 1024, 1203]     # CLS-query column per kt
BANDC = [None, 578, 757, 1025, 1204]   # band start column per kt
ES_W = 1294
AF = mybir.ActivationFunctionType
ALU = mybir.AluOpType

VCH = [(0, 512, 0, 4), (512, 256, 4, 2)]   # c0, csz, pair-group start g0, gn
PCH = [(0, 512), (512, 256)]               # proj chunks


def _build_mask_np():
    img = 24
    p = np.arange(img * img)
    pi, pj = p // img, p % img
    ok = (np.abs(pi[:, None] - pi[None, :]) <= 1) & (
        np.abs(pj[:, None] - pj[None, :]) <= 1
    )
    m = np.zeros((N, N), dtype=bool)
    m[1:, 1:] = ok
    m[0, :] = True
    m[:, 0] = True
    return m


def _build_mask_es():
    mnp = _build_mask_np().astype(np.float32)   # [577,577], symmetric
    mes = np.zeros((P, ES_W), np.float32)
    mes[:, 0:N] = mnp[0:P, 0:N]
    for kt in (1, 2, 3, 4):
        k0, ksz = KT[kt]
        q0b, bw = BAND[kt - 1]
        mes[0:ksz, Q0C[kt]] = mnp[k0 : k0 + ksz, 0]
        mes[0:ksz, BANDC[kt] : BANDC[kt] + bw] = mnp[k0 : k0 + ksz, q0b : q0b + bw]
    return mes


def _bcast_ap(ap1d, parts):
    """1-row AP -> [parts, n] with partition stride 0 (DRAM-source DMA)."""
    return bass.AP(
        tensor=ap1d.tensor, offset=ap1d.offset, ap=[[0, parts]] + list(ap1d.ap)[-1:]
    )


def _build_program():
    nc = bacc.Bacc("TRN2", target_bir_lowering=False, debug=False)

    xT = nc.dram_tensor("xT", [C, TP], BF16, kind="ExternalInput").ap()
    wqkT = nc.dram_tensor("wqkT", [C, 2 * C], BF16, kind="ExternalInput").ap()
    wvT = nc.dram_tensor("wvT", [C, C], BF16, kind="ExternalInput").ap()
    wpT = nc.dram_tensor("wpT", [C, C], BF16, kind="ExternalInput").ap()
    bqk = nc.dram_tensor("bqk", [2 * C], F32, kind="ExternalInput").ap()
    bv780 = nc.dram_tensor("bv780", [780], BF16, kind="ExternalInput").ap()
    bp = nc.dram_tensor("bp", [C], BF16, kind="ExternalInput").ap()
    maskes = nc.dram_tensor("maskes", [P, ES_W], BF16, kind="ExternalInput").ap()
    y = nc.dram_tensor("y", [T, C], BF16, kind="ExternalOutput").ap()

    with TileContext(nc) as tc:
        with (
            tc.tile_pool(name="singles", bufs=1) as singles,
            tc.tile_pool(name="xp", bufs=2) as xp,
            tc.tile_pool(name="qkp", bufs=2) as qkp,
            tc.tile_pool(name="vtp", bufs=2) as vtp,
            tc.tile_pool(name="ocp", bufs=2) as ocp,
            tc.tile_pool(name="esp", bufs=3) as esp,
            tc.tile_pool(name="tmpp", bufs=3) as tmpp,
            tc.tile_pool(name="rcp", bufs=2) as rcp,
            tc.tile_pool(name="rbp", bufs=3) as rbp,
            tc.tile_pool(name="ysp", bufs=2) as ysp,
            tc.tile_pool(name="pscp", bufs=1, space="PSUM") as pscp,
            tc.tile_pool(name="pavp", bufs=1, space="PSUM") as pavp,
            tc.tile_pool(name="pqkv", bufs=1, space="PSUM") as pqkv,
            tc.tile_pool(name="pprj", bufs=1, space="PSUM") as pprj,
            tc.tile_pool(name="drp", bufs=2, space="DRAM") as drp,
        ):
            # ---- persistent loads (x0 + wqk first: the first v/qk need them) ----
            wqk_sb = []

            def load_x(b, spread=False):
                xs = []
                for ct in range(CT):
                    t = xp.tile([P, NP], BF16, tag=f"x{ct}", name=f"x{ct}")
                    eng = (
                        (nc.sync, nc.scalar, nc.gpsimd)[ct % 3] if spread else nc.sync
                    )
                    eng.dma_start(
                        t[:], xT[ct * P : (ct + 1) * P, b * NP : (b + 1) * NP]
                    )
                    xs.append(t)
                return xs

            xT_b = load_x(0, spread=True)
            for ct in range(CT):
                t = singles.tile([P, 2 * C], BF16, tag=f"wqk{ct}")
                nc.sync.dma_start(t[:], wqkT[ct * P : (ct + 1) * P, :])
                wqk_sb.append(t)

            bqk_sb = singles.tile([P, 2 * C // P], F32, tag="bqk")
            nc.sync.dma_start(bqk_sb[:], bqk.rearrange("(o p) -> p o", p=P))
            wv_sb = []
            wp_sb = []
            for ct in range(CT):
                t = singles.tile([P, C], BF16, tag=f"wv{ct}")
                nc.gpsimd.dma_start(t[:], wvT[ct * P : (ct + 1) * P, :])
                wv_sb.append(t)
            bv_sb = singles.tile([P, 780], BF16, tag="bv")
            nc.gpsimd.dma_start(bv_sb[:], _bcast_ap(bv780, P))
            mask_sb = singles.tile([P, ES_W], BF16, tag="maskes")
            nc.gpsimd.dma_start(mask_sb[:], maskes)
            for ct in range(CT):
                t = singles.tile([P, C], BF16, tag=f"wp{ct}")
                nc.scalar.dma_start(t[:], wpT[ct * P : (ct + 1) * P, :])
                wp_sb.append(t)
            bp_sb = singles.tile([P, C], BF16, tag="bp")
            nc.scalar.dma_start(bp_sb[:], _bcast_ap(bp, P))

            def emit_v(xs):
                vts = []
                for mt, (m0, msz) in enumerate(KT):
                    vt = vtp.tile([P, 780], BF16, tag=f"vt{mt}", name=f"vt{mt}")
                    nc.gpsimd.memset(
                        vt[:msz].rearrange("p (h q) -> p h q", q=65)[:, :, 64:65],
                        1.0,
                    )
                    for c0, csz, g0, gn in VCH:
                        ps = pqkv.tile([P, 512], F32, tag="pmm", name="psv")
                        for ct in range(CT):
                            nc.tensor.matmul(
                                ps[0:msz, 0:csz],
                                xs[ct][:, m0 : m0 + msz],
                                wv_sb[ct][:, c0 : c0 + csz],
                                start=(ct == 0),
                                stop=(ct == CT - 1),
                            )
                        dst = vt[:msz].rearrange("p (g s q) -> p g s q", s=2, q=65)[
                            :, g0 : g0 + gn, :, 0:64
                        ]
                        src = ps[0:msz, 0:csz].rearrange(
                            "p (g s q) -> p g s q", s=2, q=64
                        )
                        bvv = bv_sb[0:msz].rearrange("p (g s q) -> p g s q", s=2, q=65)[
                            :, g0 : g0 + gn, :, 0:64
                        ]
                        nc.vector.tensor_tensor(dst, src, bvv, ALU.add)
                    vts.append(vt)
                return vts

            def emit_qk(hp, xs):
                qt = qkp.tile([P, N], BF16, tag="qt", name="qt")
                kt_sb = qkp.tile([P, N], BF16, tag="kt", name="kt")
                for dst, ft, sc in ((qt, hp, SCALE), (kt_sb, CT + hp, 1.0)):
                    for c0, w in ((0, 512), (512, 65)):
                        ps = pqkv.tile([P, 512], F32, tag="pmm", name="psqk")
                        for ct in range(CT):
                            nc.tensor.matmul(
                                ps[:, 0:w],
                                wqk_sb[ct][:, ft * P : (ft + 1) * P],
                                xs[ct][:, c0 : c0 + w],
                                start=(ct == 0),
                                stop=(ct == CT - 1),
                            )
                        nc.scalar.activation(
                            dst[:, c0 : c0 + w],
                            ps[:, 0:w],
                            AF.Identity,
                            bias=bqk_sb[:, ft : ft + 1],
                            scale=sc,
                        )
                return qt, kt_sb

            def emit_scores(h, qt, kt_sb, psc):
                dh = 64 * (h % 2)
                for kt in (1, 2, 3, 4):
                    k0, ksz = KT[kt]
                    q0b, bw = BAND[kt - 1]
                    lhsT = kt_sb[dh : dh + 64, k0 : k0 + ksz]
                    nc.tensor.matmul(
                        psc[0:ksz, Q0C[kt] : Q0C[kt] + 1],
                        lhsT,
                        qt[dh : dh + 64, 0:1],
                        start=True,
                        stop=True,
                    )
                    nc.tensor.matmul(
                        psc[0:ksz, BANDC[kt] : BANDC[kt] + bw],
                        lhsT,
                        qt[dh : dh + 64, q0b : q0b + bw],
                        start=True,
                        stop=True,
                    )
                for c0, w in ((0, 512), (512, 65)):
                    nc.tensor.matmul(
                        psc[0:128, c0 : c0 + w],
                        kt_sb[dh : dh + 64, 0:128],
                        qt[dh : dh + 64, c0 : c0 + w],
                        start=True,
                        stop=True,
                    )

            def emit_attnv(h, es, vts, pa, pb):
                pc = 65 * (h % 2)
                mm = nc.tensor.matmul

                def vs(kt):
                    return vts[kt][0 : KT[kt][1], 65 * h : 65 * h + 65]

                for kt in (1, 2, 3, 4):
                    k0, ksz = KT[kt]
                    q0b, bw = BAND[kt - 1]
                    # CLS-query column: each key tile owns disjoint keys.
                    # kt1's q0 matmul is the first PA writer -> start=True
                    # clears the bank's has_written bits; later matmuls then
                    # overwrite-where-clear / accumulate-where-set, so any
                    # order is correct.
                    mm(
                        pa[0:65, 0:1],
                        vs(kt),
                        es[0:ksz, Q0C[kt] : Q0C[kt] + 1],
                        start=(kt == 1),
                        stop=False,
                    )
                    wa = min(bw, 512 - q0b)     # band part in PA's bank
                    mm(
                        pa[0:65, q0b : q0b + wa],
                        vs(kt),
                        es[0:ksz, BANDC[kt] : BANDC[kt] + wa],
                        start=False,
                        stop=False,
                    )
                    if bw > wa:                 # band part spilling into PB
                        mm(
                            pb[0:65, pc + 0 : pc + (bw - wa)],
                            vs(kt),
                            es[0:ksz, BANDC[kt] + wa : BANDC[kt] + bw],
                            start=(kt == 3),
                            stop=False,
                        )
                mm(pa[0:65, 0:512], vs(0), es[0:128, 0:512], start=False, stop=True)
                mm(
                    pb[0:65, pc : pc + 65],
                    vs(0),
                    es[0:128, 512:577],
                    start=False,
                    stop=True,
                )

            def emit_proj(b, ocs):
                for mt, (m0, msz) in enumerate(KT):
                    ysb = ysp.tile([P, C], BF16, tag="ysb", name="ysb")
                    for c0, csz in PCH:
                        ps = pprj.tile([P, 512], F32, tag="pprj", name="psp")
                        for ct in range(CT):
                            nc.tensor.matmul(
                                ps[0:msz, 0:csz],
                                ocs[ct][:, m0 : m0 + msz],
                                wp_sb[ct][:, c0 : c0 + csz],
                                start=(ct == 0),
                                stop=(ct == CT - 1),
                            )
                        nc.vector.tensor_tensor(
                            ysb[0:msz, c0 : c0 + csz],
                            ps[0:msz, 0:csz],
                            bp_sb[0:msz, c0 : c0 + csz],
                            ALU.add,
                        )
                    nc.sync.dma_start(
                        y[b * N + m0 : b * N + m0 + msz, :], ysb[0:msz, :]
                    )

            dnv = None
            pending_proj = None
            for b in range(BPC):
                vts = emit_v(xT_b)
                oc_sb = [
                    ocp.tile([P, N], BF16, tag=f"oc{ct}", name=f"oc{ct}")
                    for ct in range(CT)
                ]
                dn_dram = drp.tile([H, 640], BF16, tag="dn")
                rr_dram = drp.tile([H, 640], BF16, tag="rr")
                dn_sb = rcp.tile([P, 60], BF16, tag="dn_sb")
                rr_sb = rcp.tile([P, 60], BF16, tag="rr_sb")
                dnv = dn_dram[:].rearrange("h w -> (h w)").rearrange(
                    "(p f) -> p f", f=60
                )
                rrv = rr_dram[:].rearrange("h w -> (h w)").rearrange(
                    "(p f) -> p f", f=60
                )

                pb = None
                xT_next = None
                for hp in range(6):
                    qt, kt_sb = emit_qk(hp, xT_b)
                    pb = pavp.tile([65, 130], F32, tag="pb", name="pb")
                    for pt in range(2):
                        h = 2 * hp + pt
                        psc = pscp.tile([P, ES_W], F32, tag="psc", name="psc")
                        emit_scores(h, qt, kt_sb, psc)
                        es = esp.tile([P, ES_W], BF16, tag="es", name="es")
                        nc.scalar.activation(es[:, :], psc[:, :], AF.Exp)
                        nc.vector.tensor_tensor(
                            es[:, :], es[:, :], mask_sb[:, :], ALU.mult
                        )
                        pa = pavp.tile([65, 512], F32, tag="pa", name="pa", bufs=2)
                        emit_attnv(h, es, vts, pa, pb)
                        tmp = tmpp.tile([65, N], BF16, tag="tmp", name="tmp")
                        nc.vector.tensor_copy(tmp[0:65, 0:512], pa[0:65, 0:512])
                        pc = 65 * pt
                        nc.vector.tensor_copy(
                            tmp[0:65, 512:577], pb[0:65, pc : pc + 65]
                        )
                        nc.sync.dma_start(dn_dram[h : h + 1, 0:N], tmp[64:65, :])
                        nc.sync.dma_start(
                            oc_sb[hp][64 * pt : 64 * pt + 64, :], tmp[0:64, :]
                        )

                    if hp == 3 and b + 1 < BPC:
                        xT_next = load_x(b + 1)

                    if hp in (2, 5):
                        half = hp // 3
                        p0 = 64 * half
                        nc.gpsimd.dma_start(
                            dn_sb[p0 : p0 + 64, :], dnv[p0 : p0 + 64, :]
                        )
                        with nc.allow_low_precision(
                            reason="bf16 softmax denominators; 2e-2 tolerance"
                        ):
                            nc.vector.reciprocal(
                                rr_sb[p0 : p0 + 64, :], dn_sb[p0 : p0 + 64, :]
                            )
                        nc.gpsimd.dma_start(
                            rrv[p0 : p0 + 64, :], rr_sb[p0 : p0 + 64, :]
                        )
                        for h2 in range(6 * half, 6 * half + 6):
                            rb = rbp.tile([P, N], BF16, tag="rb", name="rb")
                            r0 = 64 * (h2 % 2)
                            nc.gpsimd.dma_start(
                                rb[r0 : r0 + 64, :],
                                _bcast_ap(rr_dram[h2 : h2 + 1, 0:N], 64),
                            )
                            oc_t = oc_sb[h2 // 2]
                            nc.vector.tensor_tensor(
                                oc_t[r0 : r0 + 64, :],
                                oc_t[r0 : r0 + 64, :],
                                rb[r0 : r0 + 64, :],
                                ALU.mult,
                            )

                if pending_proj is not None:
                    emit_proj(*pending_proj)
                pending_proj = (b, oc_sb)

                if xT_next is not None:
                    xT_b = xT_next

            emit_proj(*pending_proj)

    nc.finalize()
    return nc


_CACHE = {}


def _make_in_maps(x, qkv_w, qkv_b, proj_w, proj_b):
    bf = ml_dtypes.bfloat16
    x = np.asarray(x, np.float32)
    qkv_w = np.asarray(qkv_w, np.float32)
    qkv_b = np.asarray(qkv_b, np.float32)
    proj_w = np.asarray(proj_w, np.float32)
    proj_b = np.asarray(proj_b, np.float32)

    wqkT = np.ascontiguousarray(qkv_w[: 2 * C].T).astype(bf)
    wvT = np.ascontiguousarray(qkv_w[2 * C :].T).astype(bf)
    wpT = np.ascontiguousarray(proj_w.T).astype(bf)
    bqk_h = qkv_b[: 2 * C].copy()
    bqk_h[:C] *= SCALE
    bv = qkv_b[2 * C :]
    bv780 = np.zeros(780, np.float32)
    for h in range(H):
        bv780[65 * h : 65 * h + 64] = bv[64 * h : 64 * h + 64]
    maskes = _build_mask_es().astype(bf)

    in_maps = []
    for c in range(NCORES):
        xp_c = np.zeros((BPC, NP, C), np.float32)
        xp_c[:, :N, :] = x[c * BPC : (c + 1) * BPC]
        xT_c = np.ascontiguousarray(xp_c.reshape(TP, C).T).astype(bf)
        in_maps.append(
            {
                "xT": xT_c,
                "wqkT": wqkT,
                "wvT": wvT,
                "wpT": wpT,
                "bqk": bqk_h,
                "bv780": bv780.astype(bf),
                "bp": proj_b.astype(bf),
                "maskes": maskes,
            }
        )
    return in_maps


def kernel(x, qkv_w, qkv_b, proj_w, proj_b):
    if "nc" not in _CACHE:
        _CACHE["nc"] = _build_program()
    nc = _CACHE["nc"]

    in_maps = _make_in_maps(x, qkv_w, qkv_b, proj_w, proj_b)
    res = run_bass_kernel_spmd(nc, in_maps, list(range(NCORES)))
    out = np.concatenate(
        [
            np.asarray(res.results[c]["y"]).astype(np.float32).reshape(BPC, N, C)
            for c in range(NCORES)
        ],
        axis=0,
    )
    return out
